# revision 7
# baseline (speedup 1.0000x reference)
"""Trainium2 Bass kernel for nn_CCSequenceModel (2-layer GRU encoder + autoregressive
2-layer GRU decoder with gated output head).

Strategy: pure data parallel over 8 NeuronCores (batch 8192 -> 1024/core).
On-chip layout: gate/hidden dim on partitions, batch on the free dim, processed
as two 512-sample halves (PSUM bank limit). All SBUF tensors bf16, PSUM fp32.

Per GRU cell (input dim D, hidden 64), with the cell's h living in a fixed
64-partition block ("blend side") of its stream tile and r/hn on the other block:
  P1 = W1^T @ S   (128,M) psum  -> cols arranged [z | r] so z lands blend-side
  P2 = W2^T @ S   (128,M) psum  -> [in | hn], in on blend side, hn on r side
  rz = sigmoid(P1 + b_rz)                       (ACT, per-partition bias)
  rhn = (P2[r-side] + b_hn) * rz[r-side]        (DVE scalar_tensor_tensor)
  P2[blend] += I^T @ rhn                        (PE identity accumulation)
  n = tanh(P2[blend] + b_in)                    (ACT)
  u = h - n ; u = z*u ; h' = n + u              (DVE tensor_tensor)
Biases ride in ACT bias / STT scalar operands; weights are pre-packed host-side.
x is pre-transposed host-side to (L, 6, B_core) bf16 and DMA'd per step into the
layer-0 stream tile. Decoder output cv is DMA'd per step to a DRAM staging
buffer (T, B_core); the host does the final (t,b)->(b,t) transpose.
"""
import sys
import numpy as np

for _p in ('/opt/trn_rl_repo', '/root/.axon_site/_ro/trn_rl_repo'):
    if _p not in sys.path:
        sys.path.insert(0, _p)

import ml_dtypes
import concourse.bass as bass
import concourse.tile as tile
from concourse import bacc, mybir
from concourse.bass_utils import run_bass_kernel_spmd

BF16 = mybir.dt.bfloat16
F32 = mybir.dt.float32
NPBF = ml_dtypes.bfloat16
ALU = mybir.AluOpType
ACTF = mybir.ActivationFunctionType

H = 64
NIN = 6
NCORES = 8
T_OUT = 180  # decoder length (fixed by the model)

_BUILD_CACHE = {}


# ------------------------------------------------------------------ host prep
def _pack_cell(Wih, Whh, bih, bhh, in_rows, h_rows, blend_lo, K):
    """Pack one GRU cell's weights into stationary matrices + bias vectors.

    in_rows/h_rows: slices of the stream-tile partition range carrying the
    cell input x and hidden h. blend_lo: True if the cell's blend block is
    partitions 0:64 (z/in on cols 0:64, r/hn on cols 64:128).
    Returns W1 (K,128), W2 (K,128) float32, b1 (128,), b2 (128,).
    """
    Wih = np.asarray(Wih, np.float32)
    Whh = np.asarray(Whh, np.float32)
    bih = np.asarray(bih, np.float32)
    bhh = np.asarray(bhh, np.float32)
    W1 = np.zeros((K, 128), np.float32)
    W2 = np.zeros((K, 128), np.float32)
    b1 = np.zeros(128, np.float32)
    b2 = np.zeros(128, np.float32)
    r, z, n = slice(0, 64), slice(64, 128), slice(128, 192)
    lo, hi = slice(0, 64), slice(64, 128)
    zc, rc = (lo, hi) if blend_lo else (hi, lo)   # z on blend side, r opposite
    inc, hnc = (lo, hi) if blend_lo else (hi, lo)
    # W1: r and z gates
    W1[in_rows, zc] = Wih[z].T
    W1[h_rows, zc] = Whh[z].T
    W1[in_rows, rc] = Wih[r].T
    W1[h_rows, rc] = Whh[r].T
    # W2: in (x part of n-gate) on blend side, hn (h part) on r side
    W2[in_rows, inc] = Wih[n].T
    W2[h_rows, hnc] = Whh[n].T
    b1[zc] = bih[z] + bhh[z]
    b1[rc] = bih[r] + bhh[r]
    b2[inc] = bih[n]   # tanh bias (blend side)
    b2[hnc] = bhh[n]   # hn bias (r side, applied inside STT)
    return W1, W2, b1, b2


def _prep(inputs, BC):
    """Host-side packing of all weights -> dict of replicated device arrays."""
    g = lambda k: np.asarray(inputs[k])
    out = {}
    # encoder L0: stream [h0@0:64 ; x@64:70], blend LO, K=70
    W1, W2, b1, b2 = _pack_cell(g('enc0_Wih'), g('enc0_Whh'), g('enc0_bih'),
                                g('enc0_bhh'), slice(64, 70), slice(0, 64),
                                True, 70)
    out['w1_e0'], out['w2_e0'] = W1.astype(NPBF), W2.astype(NPBF)
    be0_1, be0_2 = b1, b2
    # encoder L1: stream [h0@0:64 ; h1@64:128], blend HI, K=128
    W1, W2, b1, b2 = _pack_cell(g('enc1_Wih'), g('enc1_Whh'), g('enc1_bih'),
                                g('enc1_bhh'), slice(0, 64), slice(64, 128),
                                False, 128)
    out['w1_e1'], out['w2_e1'] = W1.astype(NPBF), W2.astype(NPBF)
    be1_1, be1_2 = b1, b2
    # decoder L0: stream [h0d@0:64 ; cv@64:65], blend LO, K=65
    W1, W2, b1, b2 = _pack_cell(g('dec0_Wih'), g('dec0_Whh'), g('dec0_bih'),
                                g('dec0_bhh'), slice(64, 65), slice(0, 64),
                                True, 65)
    out['w1_d0'], out['w2_d0'] = W1.astype(NPBF), W2.astype(NPBF)
    bd0_1, bd0_2 = b1, b2
    # decoder L1: blend HI, K=128
    W1, W2, b1, b2 = _pack_cell(g('dec1_Wih'), g('dec1_Whh'), g('dec1_bih'),
                                g('dec1_bhh'), slice(0, 64), slice(64, 128),
                                False, 128)
    out['w1_d1'], out['w2_d1'] = W1.astype(NPBF), W2.astype(NPBF)
    bd1_1, bd1_2 = b1, b2
    # heads: stationary rows are h1d (stream partitions 64:128)
    won = np.zeros((64, 1), np.float32)
    won[:, 0] = g('on_w')[0]
    wcv = np.zeros((64, 1), np.float32)
    wcv[:, 0] = g('cv_w')[0]
    out['w_on'], out['w_cv'] = won.astype(NPBF), wcv.astype(NPBF)
    # bias pack (128, 10): cols 0..7 = cell biases, col 8 = -on_b @row64,
    # col 9 = cv_b @row64
    bias = np.zeros((128, 10), np.float32)
    for j, b in enumerate([be0_1, be0_2, be1_1, be1_2,
                           bd0_1, bd0_2, bd1_1, bd1_2]):
        bias[:, j] = b
    bias[64, 8] = -float(g('on_b')[0])
    bias[64, 9] = float(g('cv_b')[0])
    out['biases'] = bias
    # identity for PE accumulation (both row halves hold I64)
    ident = np.zeros((128, 64), np.float32)
    ident[0:64] = np.eye(64)
    ident[64:128] = np.eye(64)
    out['ident'] = ident.astype(NPBF)
    return out


# ------------------------------------------------------------------ device build
def _build(L, T, BC):
    M = BC // 2
    nc = bacc.Bacc("TRN2", target_bir_lowering=False, debug=False,
                   num_devices=NCORES)
    dram = {}
    for name, shape, dt in [
        ('xT', [L, NIN, BC], BF16),
        ('w1_e0', [70, 128], BF16), ('w2_e0', [70, 128], BF16),
        ('w1_e1', [128, 128], BF16), ('w2_e1', [128, 128], BF16),
        ('w1_d0', [65, 128], BF16), ('w2_d0', [65, 128], BF16),
        ('w1_d1', [128, 128], BF16), ('w2_d1', [128, 128], BF16),
        ('w_on', [64, 1], BF16), ('w_cv', [64, 1], BF16),
        ('biases', [128, 10], F32), ('ident', [128, 64], BF16),
    ]:
        dram[name] = nc.dram_tensor(name, shape, dt, kind="ExternalInput").ap()
    stg = nc.dram_tensor("stg", [T, BC], BF16, kind="ExternalOutput").ap()

    LO, HI = slice(0, 64), slice(64, 128)

    with tile.TileContext(nc) as tc:
        const = tc.alloc_tile_pool(name="const", bufs=1)
        work = tc.alloc_tile_pool(name="work", bufs=2)

        # ---- constants into SBUF
        cw = {}
        for name in ['w1_e0', 'w2_e0', 'w1_e1', 'w2_e1', 'w1_d0', 'w2_d0',
                     'w1_d1', 'w2_d1']:
            t_ = const.tile(list(dram[name].shape), BF16, name=f"c_{name}")
            nc.sync.dma_start(out=t_, in_=dram[name])
            cw[name] = t_
        whead = const.tile([128, 2], BF16, name="c_whead")
        nc.sync.dma_start(out=whead[64:128, 0:1], in_=dram['w_on'])
        nc.sync.dma_start(out=whead[64:128, 1:2], in_=dram['w_cv'])
        bias = const.tile([128, 10], F32, name="c_bias")
        nc.sync.dma_start(out=bias, in_=dram['biases'])
        ident = const.tile([128, 64], BF16, name="c_ident")
        nc.sync.dma_start(out=ident, in_=dram['ident'])

        bcol = lambda j: bias[:, j:j + 1]

        # ---- persistent stream tiles
        s0 = [const.tile([70, BC], BF16, name=f"s0_{i}") for i in range(3)]
        s1 = [const.tile([128, BC], BF16, name=f"s1_{i}") for i in range(2)]
        sd0 = [const.tile([65, BC], BF16, name=f"sd0_{i}") for i in range(2)]
        sd1 = [const.tile([128, BC], BF16, name=f"sd1_{i}") for i in range(2)]

        # init: h0 = h1 = 0; x[0] loaded
        nc.vector.memset(s0[0][LO, :], 0.0)
        nc.vector.memset(s1[0][HI, :], 0.0)
        nc.sync.dma_start(out=s0[0][64:70, :], in_=dram['xT'][0])

        def halves(ap):
            return (ap[:, 0:M], ap[:, M:2 * M])

        def cell(pool, S, w1, w2, b1c, b2c, blend_lo, tag, ptag):
            """Emit one GRU cell step. PSUM tiles are (128, BC) = 2 banks with
            the batch halves side by side so ACT/DVE ops run at FD=BC."""
            bl, rs = (LO, HI) if blend_lo else (HI, LO)
            tp_acc = (64, 0) if blend_lo else (0, 64)
            p1 = pool.tile([128, BC], F32, name=f"p1_{tag}", tag=f"{ptag}p1")
            p2 = pool.tile([128, BC], F32, name=f"p2_{tag}", tag=f"{ptag}p2")
            rz = work.tile([128, BC], BF16, name=f"rz_{tag}", tag="rz")
            rhn = work.tile([128, BC], BF16, name=f"rhn_{tag}", tag="rhn")
            n_t = work.tile([128, BC], BF16, name=f"n_{tag}", tag="n")
            u = work.tile([128, BC], BF16, name=f"u_{tag}", tag="u")
            Sh = halves(S)
            p1h, p2h = halves(p1), halves(p2)
            for h in range(2):
                nc.tensor.matmul(p1h[h], w1, Sh[h], start=True, stop=True)
                nc.tensor.matmul(p2h[h], w2, Sh[h], start=True, stop=True)
            nc.scalar.activation(out=rz, in_=p1, func=ACTF.Sigmoid,
                                 bias=b1c, scale=1.0)
            nc.vector.scalar_tensor_tensor(
                out=rhn[rs, :], in0=p2[rs, :], scalar=b2c[rs, :],
                in1=rz[rs, :], op0=ALU.add, op1=ALU.mult)
            rhnh = halves(rhn)
            for h in range(2):
                nc.tensor.matmul(p2h[h][bl, :], ident[rs, :], rhnh[h][rs, :],
                                 start=False, stop=True, tile_position=tp_acc)
            nc.scalar.activation(out=n_t[bl, :], in_=p2[bl, :],
                                 func=ACTF.Tanh, bias=b2c[bl, :], scale=1.0)
            nc.gpsimd.tensor_tensor(out=u[bl, :], in0=S[bl, :], in1=n_t[bl, :],
                                    op=ALU.subtract)
            nc.vector.tensor_tensor(out=u[bl, :], in0=rz[bl, :], in1=u[bl, :],
                                    op=ALU.mult)
            return n_t, u, bl

        def blend_out(n_t, u, bl, out_ap):
            nc.vector.tensor_tensor(out=out_ap, in0=n_t[bl, :], in1=u[bl, :],
                                    op=ALU.add)

        # ================= encoder =================
        # separate psum tags per layer (8 banks total) so step t+1's L0
        # matmuls can overlap step t's L1 elementwise work
        eps = tc.alloc_tile_pool(name="eps", bufs=1, space="PSUM")
        for t in range(L):
            S0, S0n = s0[t % 3], s0[(t + 1) % 3]
            S1, S1n = s1[t % 2], s1[(t + 1) % 2]
            if t + 1 < L:
                nc.sync.dma_start(out=S0n[64:70, :], in_=dram['xT'][t + 1])
            # L0 -> h0' into S1 (consumed by L1 this step)
            n_t, u, bl = cell(eps, S0, cw['w1_e0'], cw['w2_e0'], bcol(0),
                              bcol(1), True, f"e0_{t}", "l0")
            blend_out(n_t, u, bl, S1[LO, :])
            # copy h0' for L0's next step
            nc.gpsimd.tensor_copy(S0n[LO, :], S1[LO, :])
            # L1 -> h1' into S1n
            n_t, u, bl = cell(eps, S1, cw['w1_e1'], cw['w2_e1'], bcol(2),
                              bcol(3), False, f"e1_{t}", "l1")
            blend_out(n_t, u, bl, S1n[HI, :])

        # ================= transition =================
        # decoder h0 init = encoder final h0 (in s1[(L-1)%2][LO]); h1 init =
        # encoder final h1 (in s1[L%2][HI]); cv init = 0
        nc.vector.tensor_copy(sd0[0][LO, :], s1[(L - 1) % 2][LO, :])
        nc.vector.tensor_copy(sd1[0][HI, :], s1[L % 2][HI, :])
        nc.vector.memset(sd0[0][64:65, :], 0.0)
        eps.release()

        # ================= decoder =================
        dps = tc.alloc_tile_pool(name="dps", bufs=1, space="PSUM")
        for t in range(T):
            D0, D0n = sd0[t % 2], sd0[(t + 1) % 2]
            D1, D1n = sd1[t % 2], sd1[(t + 1) % 2]
            n_t, u, bl = cell(dps, D0, cw['w1_d0'], cw['w2_d0'], bcol(4),
                              bcol(5), True, f"d0_{t}", "d")
            blend_out(n_t, u, bl, D1[LO, :])
            nc.gpsimd.tensor_copy(D0n[LO, :], D1[LO, :])
            n_t, u, bl = cell(dps, D1, cw['w1_d1'], cw['w2_d1'], bcol(6),
                              bcol(7), False, f"d1_{t}", "d")
            blend_out(n_t, u, bl, D1n[HI, :])
            # heads on h1' (= D1n[HI]); psum tiles live at partition 64
            cvsb = work.tile([66, BC], BF16, name=f"cvsb_{t}", tag="cvsb")
            pon = dps.tile([65, BC], F32, name=f"pon_{t}", tag="pon")
            pcv = dps.tile([65, BC], F32, name=f"pcv_{t}", tag="pcv")
            h1h = halves(D1n)
            ponh, pcvh = halves(pon), halves(pcv)
            for h in range(2):
                nc.tensor.matmul(ponh[h][64:65, :], whead[64:128, 0:1],
                                 h1h[h][HI, :], start=True, stop=True,
                                 tile_position=(64, 64))
                nc.tensor.matmul(pcvh[h][64:65, :], whead[64:128, 1:2],
                                 h1h[h][HI, :], start=True, stop=True,
                                 tile_position=(64, 64))
            # cvsb = y_cv + cv_b
            nc.vector.tensor_scalar_add(out=cvsb[64:65, :], in0=pcv[64:65, :],
                                        scalar1=bias[64:65, 9:10])
            # cv = (y_on + on_b > 0) * cvsb
            nc.vector.scalar_tensor_tensor(
                out=D0n[64:65, :], in0=pon[64:65, :], scalar=bias[64:65, 8:9],
                in1=cvsb[64:65, :], op0=ALU.is_gt, op1=ALU.mult)
            nc.gpsimd.dma_start(out=stg[t:t + 1, :], in_=D0n[64:65, :])

        dps.release()
        work.release()
        const.release()

    nc.compile()
    return nc


def _get_nc(L, T, BC):
    key = (L, T, BC)
    if key not in _BUILD_CACHE:
        _BUILD_CACHE[key] = _build(L, T, BC)
    return _BUILD_CACHE[key]


# ------------------------------------------------------------------ entry point
def kernel(**inputs):
    x = np.asarray(inputs['x'])
    B, L, _ = x.shape
    T = T_OUT
    BC = B // NCORES
    nc = _get_nc(L, T, BC)

    packed = _prep(inputs, BC)
    in_maps = []
    for c in range(NCORES):
        xs = x[c * BC:(c + 1) * BC].astype(np.float32)      # (BC, L, 6)
        xT = np.ascontiguousarray(xs.transpose(1, 2, 0)).astype(NPBF)
        m = dict(packed)
        m['xT'] = xT
        in_maps.append(m)

    res = run_bass_kernel_spmd(nc, in_maps, core_ids=list(range(NCORES)))
    out = np.empty((B, T, 1), np.float32)
    for c in range(NCORES):
        stg = np.asarray(res.results[c]['stg'], np.float32)  # (T, BC)
        out[c * BC:(c + 1) * BC, :, 0] = stg.T
    return out


# revision 11
# speedup vs baseline: 1.2825x; 1.2825x over previous
"""Trainium2 Bass kernel for nn_CCSequenceModel (2-layer GRU encoder + autoregressive
2-layer GRU decoder with gated output head).

Strategy: pure data parallel over 8 NeuronCores (batch 8192 -> 1024/core).
On-chip layout: gate/hidden dim on partitions, batch on the free dim, processed
as two 512-sample halves (PSUM bank limit). All SBUF tensors bf16, PSUM fp32.

Per GRU cell (input dim D, hidden 64), with the cell's h living in a fixed
64-partition block ("blend side") of its stream tile and r/hn on the other block:
  P1 = W1^T @ S   (128,M) psum  -> cols arranged [z | r] so z lands blend-side
  P2 = W2^T @ S   (128,M) psum  -> [in | hn], in on blend side, hn on r side
  rz = sigmoid(P1 + b_rz)                       (ACT, per-partition bias)
  rhn = (P2[r-side] + b_hn) * rz[r-side]        (DVE scalar_tensor_tensor)
  P2[blend] += I^T @ rhn                        (PE identity accumulation)
  n = tanh(P2[blend] + b_in)                    (ACT)
  u = h - n ; u = z*u ; h' = n + u              (DVE tensor_tensor)
Biases ride in ACT bias / STT scalar operands; weights are pre-packed host-side.
x is pre-transposed host-side to (L, 6, B_core) bf16 and DMA'd per step into the
layer-0 stream tile. Decoder output cv is DMA'd per step to a DRAM staging
buffer (T, B_core); the host does the final (t,b)->(b,t) transpose.
"""
import sys
import numpy as np

for _p in ('/opt/trn_rl_repo', '/root/.axon_site/_ro/trn_rl_repo'):
    if _p not in sys.path:
        sys.path.insert(0, _p)

import ml_dtypes
import concourse.bass as bass
import concourse.tile as tile
from concourse import bacc, mybir
from concourse.bass_utils import run_bass_kernel_spmd

BF16 = mybir.dt.bfloat16
F32 = mybir.dt.float32
NPBF = ml_dtypes.bfloat16
ALU = mybir.AluOpType
ACTF = mybir.ActivationFunctionType

H = 64
NIN = 6
NCORES = 8
T_OUT = 180  # decoder length (fixed by the model)

_BUILD_CACHE = {}


# ------------------------------------------------------------------ host prep
def _pack_cell(Wih, Whh, bih, bhh, in_rows, h_rows, blend_lo, K):
    """Pack one GRU cell's weights into stationary matrices + bias vectors.

    in_rows/h_rows: slices of the stream-tile partition range carrying the
    cell input x and hidden h. blend_lo: True if the cell's blend block is
    partitions 0:64 (z/in on cols 0:64, r/hn on cols 64:128).
    Returns W1 (K,128), W2 (K,128) float32, b1 (128,), b2 (128,).
    """
    Wih = np.asarray(Wih, np.float32)
    Whh = np.asarray(Whh, np.float32)
    bih = np.asarray(bih, np.float32)
    bhh = np.asarray(bhh, np.float32)
    W1 = np.zeros((K, 128), np.float32)
    W2 = np.zeros((K, 128), np.float32)
    b1 = np.zeros(128, np.float32)
    b2 = np.zeros(128, np.float32)
    r, z, n = slice(0, 64), slice(64, 128), slice(128, 192)
    lo, hi = slice(0, 64), slice(64, 128)
    zc, rc = (lo, hi) if blend_lo else (hi, lo)   # z on blend side, r opposite
    inc, hnc = (lo, hi) if blend_lo else (hi, lo)
    # W1: r and z gates
    W1[in_rows, zc] = Wih[z].T
    W1[h_rows, zc] = Whh[z].T
    W1[in_rows, rc] = Wih[r].T
    W1[h_rows, rc] = Whh[r].T
    # W2: in (x part of n-gate) on blend side, hn (h part) on r side
    W2[in_rows, inc] = Wih[n].T
    W2[h_rows, hnc] = Whh[n].T
    b1[zc] = bih[z] + bhh[z]
    b1[rc] = bih[r] + bhh[r]
    b2[inc] = bih[n]   # tanh bias (blend side)
    b2[hnc] = bhh[n]   # hn bias (r side, applied inside STT)
    return W1, W2, b1, b2


def _prep(inputs, BC):
    """Host-side packing of all weights -> dict of replicated device arrays."""
    g = lambda k: np.asarray(inputs[k])
    out = {}
    # encoder L0: stream [h0@0:64 ; x@64:70], blend LO, K=70
    W1, W2, b1, b2 = _pack_cell(g('enc0_Wih'), g('enc0_Whh'), g('enc0_bih'),
                                g('enc0_bhh'), slice(64, 70), slice(0, 64),
                                True, 70)
    out['w1_e0'], out['w2_e0'] = W1.astype(NPBF), W2.astype(NPBF)
    be0_1, be0_2 = b1, b2
    # encoder L1: stream [h0@0:64 ; h1@64:128], blend HI, K=128
    W1, W2, b1, b2 = _pack_cell(g('enc1_Wih'), g('enc1_Whh'), g('enc1_bih'),
                                g('enc1_bhh'), slice(0, 64), slice(64, 128),
                                False, 128)
    out['w1_e1'], out['w2_e1'] = W1.astype(NPBF), W2.astype(NPBF)
    be1_1, be1_2 = b1, b2
    # decoder L0: stream [h0d@0:64 ; cv@64:65], blend LO, K=65
    W1, W2, b1, b2 = _pack_cell(g('dec0_Wih'), g('dec0_Whh'), g('dec0_bih'),
                                g('dec0_bhh'), slice(64, 65), slice(0, 64),
                                True, 65)
    out['w1_d0'], out['w2_d0'] = W1.astype(NPBF), W2.astype(NPBF)
    bd0_1, bd0_2 = b1, b2
    # decoder L1: blend HI, K=128
    W1, W2, b1, b2 = _pack_cell(g('dec1_Wih'), g('dec1_Whh'), g('dec1_bih'),
                                g('dec1_bhh'), slice(0, 64), slice(64, 128),
                                False, 128)
    out['w1_d1'], out['w2_d1'] = W1.astype(NPBF), W2.astype(NPBF)
    bd1_1, bd1_2 = b1, b2
    # heads: stationary rows are h1d (stream partitions 64:128)
    won = np.zeros((64, 1), np.float32)
    won[:, 0] = g('on_w')[0]
    wcv = np.zeros((64, 1), np.float32)
    wcv[:, 0] = g('cv_w')[0]
    out['w_on'], out['w_cv'] = won.astype(NPBF), wcv.astype(NPBF)
    # bias pack (128, 10): cols 0..7 = cell biases, col 8 = -on_b @row64,
    # col 9 = cv_b @row64
    bias = np.zeros((128, 10), np.float32)
    for j, b in enumerate([be0_1, be0_2, be1_1, be1_2,
                           bd0_1, bd0_2, bd1_1, bd1_2]):
        bias[:, j] = b
    bias[64, 8] = -float(g('on_b')[0])
    bias[64, 9] = float(g('cv_b')[0])
    out['biases'] = bias
    # identity for PE accumulation (both row halves hold I64)
    ident = np.zeros((128, 64), np.float32)
    ident[0:64] = np.eye(64)
    ident[64:128] = np.eye(64)
    out['ident'] = ident.astype(NPBF)
    return out


# ------------------------------------------------------------------ device build
def _build(L, T, BC):
    M = BC // 2
    nc = bacc.Bacc("TRN2", target_bir_lowering=False, debug=False,
                   num_devices=NCORES)
    dram = {}
    for name, shape, dt in [
        ('xT', [L, NIN, BC], BF16),
        ('w1_e0', [70, 128], BF16), ('w2_e0', [70, 128], BF16),
        ('w1_e1', [128, 128], BF16), ('w2_e1', [128, 128], BF16),
        ('w1_d0', [65, 128], BF16), ('w2_d0', [65, 128], BF16),
        ('w1_d1', [128, 128], BF16), ('w2_d1', [128, 128], BF16),
        ('w_on', [64, 1], BF16), ('w_cv', [64, 1], BF16),
        ('biases', [128, 10], F32), ('ident', [128, 64], BF16),
    ]:
        dram[name] = nc.dram_tensor(name, shape, dt, kind="ExternalInput").ap()
    stg = nc.dram_tensor("stg", [T, BC], BF16, kind="ExternalOutput").ap()

    LO, HI = slice(0, 64), slice(64, 128)

    with tile.TileContext(nc) as tc:
        const = tc.alloc_tile_pool(name="const", bufs=1)
        work = tc.alloc_tile_pool(name="work", bufs=2)

        # ---- constants into SBUF
        cw = {}
        for name in ['w1_e0', 'w2_e0', 'w1_e1', 'w2_e1', 'w1_d0', 'w2_d0',
                     'w1_d1', 'w2_d1']:
            t_ = const.tile(list(dram[name].shape), BF16, name=f"c_{name}")
            nc.sync.dma_start(out=t_, in_=dram[name])
            cw[name] = t_
        whead = const.tile([128, 2], BF16, name="c_whead")
        nc.sync.dma_start(out=whead[64:128, 0:1], in_=dram['w_on'])
        nc.sync.dma_start(out=whead[64:128, 1:2], in_=dram['w_cv'])
        bias = const.tile([128, 10], F32, name="c_bias")
        nc.sync.dma_start(out=bias, in_=dram['biases'])
        ident = const.tile([128, 64], BF16, name="c_ident")
        nc.sync.dma_start(out=ident, in_=dram['ident'])

        bcol = lambda j: bias[:, j:j + 1]

        # ---- persistent stream tiles
        s0 = [const.tile([70, BC], BF16, name=f"s0_{i}") for i in range(3)]
        s1 = [const.tile([128, BC], BF16, name=f"s1_{i}") for i in range(2)]
        sd0 = [const.tile([65, BC], BF16, name=f"sd0_{i}") for i in range(2)]
        sd1 = [const.tile([128, BC], BF16, name=f"sd1_{i}") for i in range(2)]

        # init: h0 = h1 = 0; x[0] loaded
        nc.vector.memset(s0[0][LO, :], 0.0)
        nc.vector.memset(s1[0][HI, :], 0.0)
        nc.sync.dma_start(out=s0[0][64:70, :], in_=dram['xT'][0])

        def halves(ap):
            return (ap[:, 0:M], ap[:, M:2 * M])

        def gates(pool, S, w1, w2, b1c, b2c, blend_lo, tag, ptag, n_out):
            """GRU gate compute through tanh: matmuls + sigmoid + rhn +
            PE accumulation + tanh. Writes n into n_out (the cell's blend-side
            64-partition slice of a shared tile); returns the rz tile."""
            bl, rs = (LO, HI) if blend_lo else (HI, LO)
            tp_acc = (64, 0) if blend_lo else (0, 64)
            p1 = pool.tile([128, BC], F32, name=f"p1_{tag}", tag=f"{ptag}p1")
            p2 = pool.tile([128, BC], F32, name=f"p2_{tag}", tag=f"{ptag}p2")
            rz = work.tile([128, BC], BF16, name=f"rz_{tag}", tag=f"rz{ptag}")
            rhn = work.tile([128, BC], BF16, name=f"rhn_{tag}", tag=f"rhn{ptag}")
            Sh = halves(S)
            p1h, p2h = halves(p1), halves(p2)
            for h in range(2):
                nc.tensor.matmul(p1h[h], w1, Sh[h], start=True, stop=True)
                nc.tensor.matmul(p2h[h], w2, Sh[h], start=True, stop=True)
            nc.scalar.activation(out=rz, in_=p1, func=ACTF.Sigmoid,
                                 bias=b1c, scale=1.0)
            nc.vector.scalar_tensor_tensor(
                out=rhn[rs, :], in0=p2[rs, :], scalar=b2c[rs, :],
                in1=rz[rs, :], op0=ALU.add, op1=ALU.mult)
            rhnh = halves(rhn)
            for h in range(2):
                nc.tensor.matmul(p2h[h][bl, :], ident[rs, :], rhnh[h][rs, :],
                                 start=False, stop=True, tile_position=tp_acc)
            nc.scalar.activation(out=n_out, in_=p2[bl, :],
                                 func=ACTF.Tanh, bias=b2c[bl, :], scale=1.0)
            return rz

        def blend(S_old, n_t, rz0, rz1, out_ap, part=slice(0, 128), tag=""):
            """h' = n + z*(h_old - n) over the partition range `part`.
            rz0/rz1 give z on LO/HI (pass None to skip that cell's v-mul)."""
            u = work.tile([128, BC], BF16, name=f"u_{tag}", tag="u")
            v = work.tile([128, BC], BF16, name=f"v_{tag}", tag="v")
            nc.vector.tensor_tensor(out=u[part, :], in0=S_old[part, :],
                                    in1=n_t[part, :], op=ALU.subtract)
            if rz0 is not None:
                nc.vector.tensor_tensor(out=v[LO, :], in0=rz0[LO, :],
                                        in1=u[LO, :], op=ALU.mult)
            if rz1 is not None:
                nc.vector.tensor_tensor(out=v[HI, :], in0=rz1[HI, :],
                                        in1=u[HI, :], op=ALU.mult)
            nc.vector.tensor_tensor(out=out_ap, in0=n_t[part, :],
                                    in1=v[part, :], op=ALU.add)

        # ================= encoder (layer-staggered) =================
        # Tick k: L0 consumes x_k and h0_{k-1}; L1 consumes [h0_{k-1};
        # h1_{k-2}] (= S1 tile). The two layer chains are independent and
        # their blends merge into joint 128-partition ops writing the next
        # S1 = [h0_k ; h1_{k-1}]. Separate psum tags (8 banks) let them
        # overlap on the PE.
        eps = tc.alloc_tile_pool(name="eps", bufs=1, space="PSUM")
        nc.vector.memset(s1[0][LO, :], 0.0)
        nc.vector.memset(s1[1][HI, :], 0.0)
        for k in range(L + 1):
            S0, S0n = s0[k % 3], s0[(k + 1) % 3]
            S1, S1n = s1[k % 2], s1[(k + 1) % 2]
            if 1 <= k + 1 < L:
                nc.sync.dma_start(out=S0n[64:70, :], in_=dram['xT'][k + 1])
            n_t = work.tile([128, BC], BF16, name=f"n_{k}", tag="n")
            rz0 = rz1 = None
            if k < L:
                rz0 = gates(eps, S0, cw['w1_e0'], cw['w2_e0'], bcol(0),
                            bcol(1), True, f"e0_{k}", "l0", n_t[LO, :])
            if k >= 1:
                rz1 = gates(eps, S1, cw['w1_e1'], cw['w2_e1'], bcol(2),
                            bcol(3), False, f"e1_{k}", "l1", n_t[HI, :])
            if rz0 is not None and rz1 is not None:
                part = slice(0, 128)
            else:
                part = LO if rz0 is not None else HI
            blend(S1, n_t, rz0, rz1, S1n[part, :], part, f"e_{k}")
            if k < L - 1:
                nc.vector.tensor_copy(S0n[LO, :], S1n[LO, :])

        # ================= transition =================
        # decoder h0 init = h0_{L-1} (in s1[L%2][LO]); h1 init = h1_{L-1}
        # (in s1[(L+1)%2][HI]); cv init = 0
        nc.vector.tensor_copy(sd0[0][LO, :], s1[L % 2][LO, :])
        nc.vector.tensor_copy(sd1[0][HI, :], s1[(L + 1) % 2][HI, :])
        nc.vector.memset(sd0[0][64:65, :], 0.0)
        eps.release()

        # ================= decoder =================
        dps = tc.alloc_tile_pool(name="dps", bufs=1, space="PSUM")
        for t in range(T):
            D0, D0n = sd0[t % 2], sd0[(t + 1) % 2]
            D1, D1n = sd1[t % 2], sd1[(t + 1) % 2]
            n_t = work.tile([128, BC], BF16, name=f"nd_{t}", tag="n")
            rz0 = gates(dps, D0, cw['w1_d0'], cw['w2_d0'], bcol(4),
                        bcol(5), True, f"d0_{t}", "d", n_t[LO, :])
            blend(D0, n_t, rz0, None, D1[LO, :], LO, f"d0_{t}")
            nc.vector.tensor_copy(D0n[LO, :], D1[LO, :])
            rz1 = gates(dps, D1, cw['w1_d1'], cw['w2_d1'], bcol(6),
                        bcol(7), False, f"d1_{t}", "d", n_t[HI, :])
            blend(D1, n_t, None, rz1, D1n[HI, :], HI, f"d1_{t}")
            # heads on h1' (= D1n[HI]); psum tiles live at partition 64
            cvsb = work.tile([66, BC], BF16, name=f"cvsb_{t}", tag="cvsb")
            pon = dps.tile([65, BC], F32, name=f"pon_{t}", tag="pon")
            pcv = dps.tile([65, BC], F32, name=f"pcv_{t}", tag="pcv")
            h1h = halves(D1n)
            ponh, pcvh = halves(pon), halves(pcv)
            for h in range(2):
                nc.tensor.matmul(ponh[h][64:65, :], whead[64:128, 0:1],
                                 h1h[h][HI, :], start=True, stop=True,
                                 tile_position=(64, 64))
                nc.tensor.matmul(pcvh[h][64:65, :], whead[64:128, 1:2],
                                 h1h[h][HI, :], start=True, stop=True,
                                 tile_position=(64, 64))
            # cvsb = y_cv + cv_b
            nc.vector.tensor_scalar_add(out=cvsb[64:65, :], in0=pcv[64:65, :],
                                        scalar1=bias[64:65, 9:10])
            # cv = (y_on + on_b > 0) * cvsb
            nc.vector.scalar_tensor_tensor(
                out=D0n[64:65, :], in0=pon[64:65, :], scalar=bias[64:65, 8:9],
                in1=cvsb[64:65, :], op0=ALU.is_gt, op1=ALU.mult)
            nc.gpsimd.dma_start(out=stg[t:t + 1, :], in_=D0n[64:65, :])

        dps.release()
        work.release()
        const.release()

    nc.compile()
    return nc


def _get_nc(L, T, BC):
    key = (L, T, BC)
    if key not in _BUILD_CACHE:
        _BUILD_CACHE[key] = _build(L, T, BC)
    return _BUILD_CACHE[key]


# ------------------------------------------------------------------ entry point
def kernel(**inputs):
    x = np.asarray(inputs['x'])
    B, L, _ = x.shape
    T = T_OUT
    BC = B // NCORES
    nc = _get_nc(L, T, BC)

    packed = _prep(inputs, BC)
    in_maps = []
    for c in range(NCORES):
        xs = x[c * BC:(c + 1) * BC].astype(np.float32)      # (BC, L, 6)
        xT = np.ascontiguousarray(xs.transpose(1, 2, 0)).astype(NPBF)
        m = dict(packed)
        m['xT'] = xT
        in_maps.append(m)

    res = run_bass_kernel_spmd(nc, in_maps, core_ids=list(range(NCORES)))
    out = np.empty((B, T, 1), np.float32)
    for c in range(NCORES):
        stg = np.asarray(res.results[c]['stg'], np.float32)  # (T, BC)
        out[c * BC:(c + 1) * BC, :, 0] = stg.T
    return out


# revision 17
# speedup vs baseline: 1.4537x; 1.1334x over previous
"""Trainium2 Bass kernel for nn_CCSequenceModel (2-layer GRU encoder + autoregressive
2-layer GRU decoder with gated output head).

Strategy: pure data parallel over 8 NeuronCores (batch 8192 -> 1024/core).
On-chip layout: gate/hidden dim on partitions, batch on the free dim, processed
as two 512-sample halves (PSUM bank limit). All SBUF tensors bf16, PSUM fp32.

Per GRU cell (input dim D, hidden 64), with the cell's h living in a fixed
64-partition block ("blend side") of its stream tile and r/hn on the other block:
  P1 = W1^T @ S   (128,M) psum  -> cols arranged [z | r] so z lands blend-side
  P2 = W2^T @ S   (128,M) psum  -> [in | hn], in on blend side, hn on r side
  rz = sigmoid(P1 + b_rz)                       (ACT, per-partition bias)
  rhn = (P2[r-side] + b_hn) * rz[r-side]        (DVE scalar_tensor_tensor)
  P2[blend] += I^T @ rhn                        (PE identity accumulation)
  n = tanh(P2[blend] + b_in)                    (ACT)
  u = h - n ; u = z*u ; h' = n + u              (DVE tensor_tensor)
Biases ride in ACT bias / STT scalar operands; weights are pre-packed host-side.
x is pre-transposed host-side to (L, 6, B_core) bf16 and DMA'd per step into the
layer-0 stream tile. Decoder output cv is DMA'd per step to a DRAM staging
buffer (T, B_core); the host does the final (t,b)->(b,t) transpose.
"""
import sys
import numpy as np

for _p in ('/opt/trn_rl_repo', '/root/.axon_site/_ro/trn_rl_repo'):
    if _p not in sys.path:
        sys.path.insert(0, _p)

import ml_dtypes
import concourse.bass as bass
import concourse.tile as tile
from concourse import bacc, mybir
from concourse.bass_utils import run_bass_kernel_spmd

BF16 = mybir.dt.bfloat16
F32 = mybir.dt.float32
NPBF = ml_dtypes.bfloat16
ALU = mybir.AluOpType
ACTF = mybir.ActivationFunctionType

H = 64
NIN = 6
NCORES = 8
T_OUT = 180  # decoder length (fixed by the model)

_BUILD_CACHE = {}


# ------------------------------------------------------------------ host prep
def _pack_cell(Wih, Whh, bih, bhh, in_rows, h_rows, blend_lo, K):
    """Pack one GRU cell's weights into stationary matrices + bias vectors.

    in_rows/h_rows: slices of the stream-tile partition range carrying the
    cell input x and hidden h. blend_lo: True if the cell's blend block is
    partitions 0:64 (z/in on cols 0:64, r/hn on cols 64:128).
    Returns W1 (K,128), W2 (K,128) float32, b1 (128,), b2 (128,).
    """
    Wih = np.asarray(Wih, np.float32)
    Whh = np.asarray(Whh, np.float32)
    bih = np.asarray(bih, np.float32)
    bhh = np.asarray(bhh, np.float32)
    W1 = np.zeros((K, 128), np.float32)
    W2 = np.zeros((K, 128), np.float32)
    b1 = np.zeros(128, np.float32)
    b2 = np.zeros(128, np.float32)
    r, z, n = slice(0, 64), slice(64, 128), slice(128, 192)
    lo, hi = slice(0, 64), slice(64, 128)
    zc, rc = (lo, hi) if blend_lo else (hi, lo)   # z on blend side, r opposite
    inc, hnc = (lo, hi) if blend_lo else (hi, lo)
    # W1: r and z gates
    W1[in_rows, zc] = Wih[z].T
    W1[h_rows, zc] = Whh[z].T
    W1[in_rows, rc] = Wih[r].T
    W1[h_rows, rc] = Whh[r].T
    # W2: in (x part of n-gate) on blend side, hn (h part) on r side
    W2[in_rows, inc] = Wih[n].T
    W2[h_rows, hnc] = Whh[n].T
    b1[zc] = bih[z] + bhh[z]
    b1[rc] = bih[r] + bhh[r]
    b2[inc] = bih[n]   # tanh bias (blend side)
    b2[hnc] = bhh[n]   # hn bias (r side, applied inside STT)
    return W1, W2, b1, b2


def _prep(inputs, BC):
    """Host-side packing of all weights -> dict of replicated device arrays."""
    g = lambda k: np.asarray(inputs[k])
    out = {}
    # encoder L0: stream [h0@0:64 ; x@64:70], blend LO, K=70
    W1, W2, b1, b2 = _pack_cell(g('enc0_Wih'), g('enc0_Whh'), g('enc0_bih'),
                                g('enc0_bhh'), slice(64, 70), slice(0, 64),
                                True, 70)
    out['w1_e0'], out['w2_e0'] = W1.astype(NPBF), W2.astype(NPBF)
    be0_1, be0_2 = b1, b2
    # encoder L1: stream [h0@0:64 ; h1@64:128], blend HI, K=128
    W1, W2, b1, b2 = _pack_cell(g('enc1_Wih'), g('enc1_Whh'), g('enc1_bih'),
                                g('enc1_bhh'), slice(0, 64), slice(64, 128),
                                False, 128)
    out['w1_e1'], out['w2_e1'] = W1.astype(NPBF), W2.astype(NPBF)
    be1_1, be1_2 = b1, b2
    # decoder L0: stream [h0d@0:64 ; cv@64:65], blend LO, K=65
    W1, W2, b1, b2 = _pack_cell(g('dec0_Wih'), g('dec0_Whh'), g('dec0_bih'),
                                g('dec0_bhh'), slice(64, 65), slice(0, 64),
                                True, 65)
    out['w1_d0'], out['w2_d0'] = W1.astype(NPBF), W2.astype(NPBF)
    bd0_1, bd0_2 = b1, b2
    # decoder L1: blend HI, K=128
    W1, W2, b1, b2 = _pack_cell(g('dec1_Wih'), g('dec1_Whh'), g('dec1_bih'),
                                g('dec1_bhh'), slice(0, 64), slice(64, 128),
                                False, 128)
    out['w1_d1'], out['w2_d1'] = W1.astype(NPBF), W2.astype(NPBF)
    bd1_1, bd1_2 = b1, b2
    # heads: stationary rows are h1d (stream partitions 64:128)
    won = np.zeros((64, 1), np.float32)
    won[:, 0] = g('on_w')[0]
    wcv = np.zeros((64, 1), np.float32)
    wcv[:, 0] = g('cv_w')[0]
    out['w_on'], out['w_cv'] = won.astype(NPBF), wcv.astype(NPBF)
    # bias pack (128, 10): cols 0..7 = cell biases, col 8 = -on_b @row64,
    # col 9 = cv_b @row64
    bias = np.zeros((128, 10), np.float32)
    for j, b in enumerate([be0_1, be0_2, be1_1, be1_2,
                           bd0_1, bd0_2, bd1_1, bd1_2]):
        bias[:, j] = b
    bias[64, 8] = -float(g('on_b')[0])
    bias[64, 9] = float(g('cv_b')[0])
    out['biases'] = bias
    # identity for PE accumulation (both row halves hold I64)
    ident = np.zeros((128, 64), np.float32)
    ident[0:64] = np.eye(64)
    ident[64:128] = np.eye(64)
    out['ident'] = ident.astype(NPBF)
    return out


# ------------------------------------------------------------------ device build
def _build(L, T, BC):
    M = BC // 2
    nc = bacc.Bacc("TRN2", target_bir_lowering=False, debug=False,
                   num_devices=NCORES)
    dram = {}
    for name, shape, dt in [
        ('xT', [L, NIN, BC], BF16),
        ('w1_e0', [70, 128], BF16), ('w2_e0', [70, 128], BF16),
        ('w1_e1', [128, 128], BF16), ('w2_e1', [128, 128], BF16),
        ('w1_d0', [65, 128], BF16), ('w2_d0', [65, 128], BF16),
        ('w1_d1', [128, 128], BF16), ('w2_d1', [128, 128], BF16),
        ('w_on', [64, 1], BF16), ('w_cv', [64, 1], BF16),
        ('biases', [128, 10], F32), ('ident', [128, 64], BF16),
    ]:
        dram[name] = nc.dram_tensor(name, shape, dt, kind="ExternalInput").ap()
    stg = nc.dram_tensor("stg", [T, BC], BF16, kind="ExternalOutput").ap()

    LO, HI = slice(0, 64), slice(64, 128)

    with tile.TileContext(nc) as tc:
        const = tc.alloc_tile_pool(name="const", bufs=1)
        work = tc.alloc_tile_pool(name="work", bufs=2)

        # ---- constants into SBUF
        cw = {}
        for name in ['w1_e0', 'w2_e0', 'w1_e1', 'w2_e1', 'w1_d0', 'w2_d0',
                     'w1_d1', 'w2_d1']:
            t_ = const.tile(list(dram[name].shape), BF16, name=f"c_{name}")
            nc.sync.dma_start(out=t_, in_=dram[name])
            cw[name] = t_
        whead = const.tile([128, 2], BF16, name="c_whead")
        nc.sync.dma_start(out=whead[64:128, 0:1], in_=dram['w_on'])
        nc.sync.dma_start(out=whead[64:128, 1:2], in_=dram['w_cv'])
        bias = const.tile([128, 10], F32, name="c_bias")
        nc.sync.dma_start(out=bias, in_=dram['biases'])
        ident = const.tile([128, 64], BF16, name="c_ident")
        nc.sync.dma_start(out=ident, in_=dram['ident'])

        bcol = lambda j: bias[:, j:j + 1]

        # ---- persistent stream tiles
        s0 = [const.tile([70, BC], BF16, name=f"s0_{i}") for i in range(3)]
        s1 = [const.tile([128, BC], BF16, name=f"s1_{i}") for i in range(2)]
        sd0 = [const.tile([65, BC], BF16, name=f"sd0_{i}") for i in range(2)]
        sd1 = [const.tile([128, BC], BF16, name=f"sd1_{i}") for i in range(2)]

        # init: h0 = h1 = 0; x[0] loaded
        nc.vector.memset(s0[0][LO, :], 0.0)
        nc.vector.memset(s1[0][HI, :], 0.0)
        nc.sync.dma_start(out=s0[0][64:70, :], in_=dram['xT'][0])

        def halves(ap):
            return (ap[:, 0:M], ap[:, M:2 * M])

        def hs(ap, h):
            return ap[:, h * M:(h + 1) * M]

        def cell_alloc(pool, tag, ptag):
            """Per-(cell, tick) tiles shared by the two batch-half chains."""
            p1 = pool.tile([128, BC], F32, name=f"p1_{tag}", tag=f"{ptag}p1")
            p2 = pool.tile([128, BC], F32, name=f"p2_{tag}", tag=f"{ptag}p2")
            rz = work.tile([128, BC], BF16, name=f"rz_{tag}", tag=f"rz{ptag}")
            rhn = work.tile([128, BC], BF16, name=f"rhn_{tag}",
                            tag=f"rhn{ptag}")
            return p1, p2, rz, rhn

        def gates_h(ct, S, h, w1, w2, b1c, b2c, blend_lo, n_t):
            """One GRU cell's gate compute (through tanh) for batch half h.
            Per-half ops keep the A/B dependency chains independent so they
            interleave on the engines. Writes sigmoid output into rz half and
            n into n_t's blend-side half."""
            p1, p2, rz, rhn = ct
            bl, rs = (LO, HI) if blend_lo else (HI, LO)
            tp_acc = (64, 0) if blend_lo else (0, 64)
            nc.tensor.matmul(hs(p1, h), w1, hs(S, h), start=True, stop=True)
            nc.tensor.matmul(hs(p2, h), w2, hs(S, h), start=True, stop=True)
            nc.scalar.activation(out=hs(rz, h), in_=hs(p1, h),
                                 func=ACTF.Sigmoid, bias=b1c, scale=1.0)
            nc.vector.scalar_tensor_tensor(
                out=hs(rhn, h)[rs, :], in0=hs(p2, h)[rs, :], scalar=b2c[rs, :],
                in1=hs(rz, h)[rs, :], op0=ALU.add, op1=ALU.mult)
            nc.tensor.matmul(hs(p2, h)[bl, :], ident[rs, :],
                             hs(rhn, h)[rs, :], start=False, stop=True,
                             tile_position=tp_acc)
            nc.scalar.activation(out=hs(n_t, h)[bl, :], in_=hs(p2, h)[bl, :],
                                 func=ACTF.Tanh, bias=b2c[bl, :], scale=1.0)

        def blend_h(S_old, h, n_t, rz0, rz1, out_ap, part, tag):
            """h' = n + z*(h_old - n) for batch half h over partitions `part`.
            rz0/rz1 supply z on LO/HI (None skips that cell)."""
            u = work.tile([128, BC], BF16, name=f"u_{tag}", tag="u")
            v = work.tile([128, BC], BF16, name=f"v_{tag}", tag="v")
            nc.vector.tensor_tensor(out=hs(u, h)[part, :],
                                    in0=hs(S_old, h)[part, :],
                                    in1=hs(n_t, h)[part, :], op=ALU.subtract)
            if rz0 is not None:
                nc.vector.tensor_tensor(out=hs(v, h)[LO, :],
                                        in0=hs(rz0, h)[LO, :],
                                        in1=hs(u, h)[LO, :], op=ALU.mult)
            if rz1 is not None:
                nc.vector.tensor_tensor(out=hs(v, h)[HI, :],
                                        in0=hs(rz1, h)[HI, :],
                                        in1=hs(u, h)[HI, :], op=ALU.mult)
            nc.vector.tensor_tensor(out=out_ap, in0=hs(n_t, h)[part, :],
                                    in1=hs(v, h)[part, :], op=ALU.add)

        # ================= encoder (layer-staggered) =================
        # Tick k: L0 consumes x_k and h0_{k-1}; L1 consumes [h0_{k-1};
        # h1_{k-2}] (= S1 tile). The two layer chains are independent and
        # their blends merge into joint 128-partition ops writing the next
        # S1 = [h0_k ; h1_{k-1}]. Separate psum tags (8 banks) let them
        # overlap on the PE.
        eps = tc.alloc_tile_pool(name="eps", bufs=1, space="PSUM")
        nc.vector.memset(s1[0][LO, :], 0.0)
        nc.vector.memset(s1[1][HI, :], 0.0)
        for k in range(L + 1):
            S0, S0n = s0[k % 3], s0[(k + 1) % 3]
            S1, S1n = s1[k % 2], s1[(k + 1) % 2]
            if 1 <= k + 1 < L:
                nc.sync.dma_start(out=S0n[64:70, :], in_=dram['xT'][k + 1])
            n_t = work.tile([128, BC], BF16, name=f"n_{k}", tag="n")
            has0, has1 = k < L, k >= 1
            if has0 and has1:
                part = slice(0, 128)
            else:
                part = LO if has0 else HI
            ct0 = cell_alloc(eps, f"e0_{k}", "l0") if has0 else None
            ct1 = cell_alloc(eps, f"e1_{k}", "l1") if has1 else None
            for h in range(2):
                if has0:
                    gates_h(ct0, S0, h, cw['w1_e0'], cw['w2_e0'], bcol(0),
                            bcol(1), True, n_t)
                if has1:
                    gates_h(ct1, S1, h, cw['w1_e1'], cw['w2_e1'], bcol(2),
                            bcol(3), False, n_t)
                blend_h(S1, h, n_t, ct0[2] if has0 else None,
                        ct1[2] if has1 else None, hs(S1n, h)[part, :],
                        part, f"e_{k}")
                if k < L - 1:
                    nc.vector.tensor_copy(hs(S0n, h)[LO, :],
                                          hs(S1n, h)[LO, :])

        # ================= transition =================
        # decoder h0 init = h0_{L-1} (in s1[L%2][LO]); h1 init = h1_{L-1}
        # (in s1[(L+1)%2][HI]); cv init = 0
        nc.vector.tensor_copy(sd0[0][LO, :], s1[L % 2][LO, :])
        nc.vector.tensor_copy(sd1[0][HI, :], s1[(L + 1) % 2][HI, :])
        nc.vector.memset(sd0[0][64:65, :], 0.0)
        eps.release()

        # ================= decoder =================
        dps = tc.alloc_tile_pool(name="dps", bufs=1, space="PSUM")
        for t in range(T):
            D0, D0n = sd0[t % 2], sd0[(t + 1) % 2]
            D1, D1n = sd1[t % 2], sd1[(t + 1) % 2]
            n_t = work.tile([128, BC], BF16, name=f"nd_{t}", tag="n")
            cvsb = work.tile([66, BC], BF16, name=f"cvsb_{t}", tag="cvsb")
            ct0 = cell_alloc(dps, f"d0_{t}", "d0")
            ct1 = cell_alloc(dps, f"d1_{t}", "d1")
            # heads psum shares the d0 tags: their lifetimes dovetail with the
            # real cv -> next-step-dec0 dependency, keeping total PSUM at 8 banks
            pon = dps.tile([65, BC], F32, name=f"pon_{t}", tag="d0p1")
            pcv = dps.tile([65, BC], F32, name=f"pcv_{t}", tag="d0p2")
            for h in range(2):
                gates_h(ct0, D0, h, cw['w1_d0'], cw['w2_d0'], bcol(4),
                        bcol(5), True, n_t)
                blend_h(D0, h, n_t, ct0[2], None, hs(D1, h)[LO, :], LO,
                        f"d0_{t}")
                nc.vector.tensor_copy(hs(D0n, h)[LO, :], hs(D1, h)[LO, :])
                gates_h(ct1, D1, h, cw['w1_d1'], cw['w2_d1'], bcol(6),
                        bcol(7), False, n_t)
                blend_h(D1, h, n_t, None, ct1[2], hs(D1n, h)[HI, :], HI,
                        f"d1_{t}")
                # heads on h1' (= D1n[HI]); psum tiles live at partition 64
                nc.tensor.matmul(hs(pon, h)[64:65, :], whead[64:128, 0:1],
                                 hs(D1n, h)[HI, :], start=True, stop=True,
                                 tile_position=(64, 64))
                nc.tensor.matmul(hs(pcv, h)[64:65, :], whead[64:128, 1:2],
                                 hs(D1n, h)[HI, :], start=True, stop=True,
                                 tile_position=(64, 64))
                # cvsb = y_cv + cv_b
                nc.vector.tensor_scalar_add(out=hs(cvsb, h)[64:65, :],
                                            in0=hs(pcv, h)[64:65, :],
                                            scalar1=bias[64:65, 9:10])
                # cv = (y_on + on_b > 0) * cvsb
                nc.vector.scalar_tensor_tensor(
                    out=hs(D0n, h)[64:65, :], in0=hs(pon, h)[64:65, :],
                    scalar=bias[64:65, 8:9], in1=hs(cvsb, h)[64:65, :],
                    op0=ALU.is_gt, op1=ALU.mult)
            nc.gpsimd.dma_start(out=stg[t:t + 1, :], in_=D0n[64:65, :])

        dps.release()
        work.release()
        const.release()

    nc.compile()
    return nc


def _get_nc(L, T, BC):
    key = (L, T, BC)
    if key not in _BUILD_CACHE:
        _BUILD_CACHE[key] = _build(L, T, BC)
    return _BUILD_CACHE[key]


# ------------------------------------------------------------------ entry point
def kernel(**inputs):
    x = np.asarray(inputs['x'])
    B, L, _ = x.shape
    T = T_OUT
    BC = B // NCORES
    nc = _get_nc(L, T, BC)

    packed = _prep(inputs, BC)
    in_maps = []
    for c in range(NCORES):
        xs = x[c * BC:(c + 1) * BC].astype(np.float32)      # (BC, L, 6)
        xT = np.ascontiguousarray(xs.transpose(1, 2, 0)).astype(NPBF)
        m = dict(packed)
        m['xT'] = xT
        in_maps.append(m)

    res = run_bass_kernel_spmd(nc, in_maps, core_ids=list(range(NCORES)))
    out = np.empty((B, T, 1), np.float32)
    for c in range(NCORES):
        stg = np.asarray(res.results[c]['stg'], np.float32)  # (T, BC)
        out[c * BC:(c + 1) * BC, :, 0] = stg.T
    return out
